# revision 12
# baseline (speedup 1.0000x reference)
"""Trainium2 Bass kernel for BertSelfAttention with C_prior multiply.

Reference (per batch b):
  q/k/v = x @ W{q,k,v}.T + b{q,k,v}            -> [S, D], split into H=16 heads of W=64
  scores = q k^T / sqrt(W); mask; softmax over k
  attn = softmax(scores) * C_prior[b]
  out = attn @ v                               -> [B, S, D]

Shapes: B=2, S=2048, D=1024, H=16, W=64.

Sharding: 8 cores; core c owns batch b=c//4 and 4 consecutive heads
(hg=c%4 -> heads 4*hg..4*hg+3). The whole per-(b,h) score block stays local.

Device pipeline (per core), v2:
  - 8 phases = (head pair pr in {0,1}) x (q block qh in {0..3} of 512).
  - Per k-strip (128 rows): the two heads of the pair are packed in the
    FREE dim of one [128, 1024] fp32 PSUM tile (2 banks): two row-tiled
    64-contraction matmuls run concurrently, then ONE 1024-wide EXP
    (halves the ScalarE instruction overhead -- ScalarE is the pipeline
    limiter at ~1 elem/cycle/lane).
  - attn*C: one 1024-wide VE bf16 multiply; the [128,512] ct strip is
    read twice via a stride-0 broadcast AP (no duplicate DMA).
  - Softmax denominator: ones(mask)-matmul pairs accumulated in one PSUM
    bank; A@V pairs in another. exp skips max-subtraction (scores~N(0,1)).
  - Projections are spread across the strip pipeline with a deadline
    schedule (one 8-matmul PSUM chain at a time in a dedicated bank);
    xT is DMA'd q-block-major so the first chains start after ~2MB.
  - Output O^T [w, q] written per phase; host transposes on gather.
"""

import os

import numpy as np
import ml_dtypes

B, S, D, H, W = 2, 2048, 1024, 16, 64
NCORES = 8
HEADS_PER_CORE = 4
P = 128
QH = 512  # q block per phase
NK = S // P  # 16 k-strips
NPH = 8  # phases: (pr, qh)
BOFF = 8  # pass B trails pass A by 8 strips
AOFF = 6  # attn*C multiply trails pass A by 6 strips

_prog_cache = {}


def _build_program():
    import concourse.mybir as mybir
    import concourse.tile as tile
    from concourse import bacc

    dt = mybir.dt
    f32, bf16 = dt.float32, dt.bfloat16
    Alu = mybir.AluOpType
    Act = mybir.ActivationFunctionType

    nc = bacc.Bacc("TRN2", target_bir_lowering=False)

    xT_d = nc.declare_dram_parameter("xT", [D, S], bf16, isOutput=False)
    wqk_d = nc.declare_dram_parameter("wqk", [D, 512], bf16, isOutput=False)
    wv_d = nc.declare_dram_parameter("wv", [D, 256], bf16, isOutput=False)
    bqk_d = nc.declare_dram_parameter("bqk", [P, 4], f32, isOutput=False)
    bvr_d = nc.declare_dram_parameter("bvr", [P, 256], f32, isOutput=False)
    ct_d = nc.declare_dram_parameter("ct", [S, S], bf16, isOutput=False)
    mk_d = nc.declare_dram_parameter("mk", [P, NK * 64], bf16, isOutput=False)
    out_d = nc.declare_dram_parameter("out", [256, S], f32, isOutput=True)

    with tile.TileContext(nc) as tc:
        with tc.tile_pool(name="persist", bufs=1) as persist:
            qk_all = persist.tile([P, 4, S], bf16)
            v_sb = persist.tile([P, NK, 256], bf16)
            bqk_sb = persist.tile([P, 4], f32)
            bvr_sb = persist.tile([P, 256], f32)
            mk_sb = persist.tile([P, NK, 64], bf16)
            xT_sb = persist.tile([P, 8, S], bf16)
            wqk_sb = persist.tile([P, 8, 512], bf16)
            wv_sb = persist.tile([P, 8, 256], bf16)
            nc.sync.dma_start(out=bqk_sb[:], in_=bqk_d[:])
            nc.sync.dma_start(out=bvr_sb[:], in_=bvr_d[:])
            nc.sync.dma_start(out=mk_sb[:], in_=mk_d[:])
            xT_r = xT_d.rearrange("(o p) q -> p o q", p=P)
            wqk_r = wqk_d.rearrange("(o p) m -> p o m", p=P)
            wv_r = wv_d.rearrange("(o p) m -> p o m", p=P)
            # critical first 2MB: wqk + xT q-block 0 (feeds the K0/Q0 chains),
            # split into 64KB pieces so the first dc chunks land a wave early
            for dc in range(8):
                nc.sync.dma_start(
                    out=wqk_sb[:, dc, 0:256], in_=wqk_r[:, dc, 0:256]
                )
                nc.sync.dma_start(
                    out=wqk_sb[:, dc, 256:512], in_=wqk_r[:, dc, 256:512]
                )
                nc.sync.dma_start(out=xT_sb[:, dc, 0:256], in_=xT_r[:, dc, 0:256])
                nc.sync.dma_start(
                    out=xT_sb[:, dc, 256:512], in_=xT_r[:, dc, 256:512]
                )
            for dc in range(8):
                nc.sync.dma_start(out=wv_sb[:, dc, :], in_=wv_r[:, dc, :])
            for qb in (1, 2, 3):
                for dc in range(8):
                    qs = slice(qb * 512, (qb + 1) * 512)
                    nc.sync.dma_start(out=xT_sb[:, dc, qs], in_=xT_r[:, dc, qs])

            with tc.tile_pool(name="estr", bufs=11) as ep, tc.tile_pool(
                name="astr", bufs=4
            ) as app, tc.tile_pool(name="ctp", bufs=9) as ctp, tc.tile_pool(
                name="small", bufs=1
            ) as smallp, tc.tile_pool(
                name="psA", bufs=2, space="PSUM"
            ) as psA, tc.tile_pool(
                name="pop", bufs=1, space="PSUM"
            ) as pop, tc.tile_pool(
                name="prsp", bufs=1, space="PSUM"
            ) as prsp, tc.tile_pool(
                name="projp", bufs=2, space="PSUM"
            ) as projp:

                def qk_step(ps, col, qs, dc):
                    # col-split: two concurrent 64-col quadrant matmuls whose
                    # weight loads overlap the neighbouring quadrant's stream
                    nc.tensor.matmul(
                        ps[0:64, :],
                        lhsT=wqk_sb[:, dc, col * P : col * P + 64],
                        rhs=xT_sb[:, dc, qs],
                        tile_position=(0, 0),
                        start=(dc == 0),
                        stop=(dc == 7),
                    )
                    nc.tensor.matmul(
                        ps[64:128, :],
                        lhsT=wqk_sb[:, dc, col * P + 64 : (col + 1) * P],
                        rhs=xT_sb[:, dc, qs],
                        tile_position=(0, 64),
                        start=(dc == 0),
                        stop=(dc == 7),
                    )

                def qk_chain(col, qb):
                    ps = projp.tile([P, 512], f32, tag="proj")
                    qs = slice(qb * 512, (qb + 1) * 512)
                    for dc in range(8):
                        qk_step(ps, col, qs, dc)
                        if dc < 7:
                            yield
                    nc.vector.tensor_scalar_add(
                        out=qk_all[:, col, qs],
                        in0=ps[:],
                        scalar1=bqk_sb[:, col : col + 1],
                    )

                def v_chain(kt):
                    ps = projp.tile([P, 512], f32, tag="proj")
                    for dc in range(8):
                        nc.tensor.matmul(
                            ps[0:64, 0:256],
                            lhsT=xT_sb[:, dc, kt * P : kt * P + 64],
                            rhs=wv_sb[:, dc, :],
                            tile_position=(0, 0),
                            start=(dc == 0),
                            stop=(dc == 7),
                        )
                        nc.tensor.matmul(
                            ps[64:128, 0:256],
                            lhsT=xT_sb[:, dc, kt * P + 64 : (kt + 1) * P],
                            rhs=wv_sb[:, dc, :],
                            tile_position=(0, 64),
                            start=(dc == 0),
                            stop=(dc == 7),
                        )
                        if dc < 7:
                            yield
                    nc.vector.tensor_tensor(
                        v_sb[:, kt, :], ps[:, 0:256], bvr_sb[:], Alu.add
                    )

                def ct_fetch(qh, ks):
                    ct = ctp.tile([P, 512], bf16, tag="ct")
                    nc.sync.dma_start(
                        out=ct[:],
                        in_=ct_d[ks * P : (ks + 1) * P, qh * QH : (qh + 1) * QH],
                    )
                    return ct

                def scores(pr, qh, ks):
                    # 4 quadrant matmuls (row x col split), all concurrent
                    pss = psA.tile([P, 1024], f32, tag="scT")
                    qs = slice(qh * QH, (qh + 1) * QH)
                    for hh, rlo in ((0, 0), (1, 64)):
                        rs = slice(rlo, rlo + 64)
                        os_ = slice(hh * 512, (hh + 1) * 512)
                        nc.tensor.matmul(
                            pss[0:64, os_],
                            lhsT=qk_all[rs, 2 * pr + 1, ks * P : ks * P + 64],
                            rhs=qk_all[rs, 2 * pr, qs],
                            tile_position=(rlo, 0),
                            start=True,
                            stop=True,
                        )
                        nc.tensor.matmul(
                            pss[64:128, os_],
                            lhsT=qk_all[rs, 2 * pr + 1, ks * P + 64 : (ks + 1) * P],
                            rhs=qk_all[rs, 2 * pr, qs],
                            tile_position=(rlo, 64),
                            start=True,
                            stop=True,
                        )
                    return pss

                def expo(pss):
                    e = ep.tile([P, 1024], bf16, tag="e")
                    nc.scalar.activation(e[:], pss[:], Act.Exp, scale=0.125)
                    return e

                def a_mult(e, ct):
                    a = app.tile([P, 1024], bf16, tag="a")
                    ct_b = ct[:, None, :].broadcast_to([P, 2, 512])
                    nc.vector.tensor_tensor(
                        a[:].rearrange("p (r f) -> p r f", r=2),
                        e[:].rearrange("p (r f) -> p r f", r=2),
                        ct_b,
                        Alu.mult,
                    )
                    return a

                def prs_mm(ks, e, prs):
                    st, sp = (ks == 0), (ks == NK - 1)
                    nc.tensor.matmul(
                        prs[0:64, :],
                        lhsT=mk_sb[:, ks, :],
                        rhs=e[:, 0:512],
                        tile_position=(0, 0),
                        start=st,
                        stop=sp,
                    )
                    nc.tensor.matmul(
                        prs[64:128, :],
                        lhsT=mk_sb[:, ks, :],
                        rhs=e[:, 512:1024],
                        tile_position=(0, 64),
                        start=st,
                        stop=sp,
                    )

                def po_mm(pr, ks, a, po):
                    st, sp = (ks == 0), (ks == NK - 1)
                    h0, h1 = 2 * pr, 2 * pr + 1
                    nc.tensor.matmul(
                        po[0:64, :],
                        lhsT=v_sb[:, ks, h0 * 64 : (h0 + 1) * 64],
                        rhs=a[:, 0:512],
                        tile_position=(0, 0),
                        start=st,
                        stop=sp,
                    )
                    nc.tensor.matmul(
                        po[64:128, :],
                        lhsT=v_sb[:, ks, h1 * 64 : (h1 + 1) * 64],
                        rhs=a[:, 512:1024],
                        tile_position=(0, 64),
                        start=st,
                        stop=sp,
                    )

                def finishB(pr, qh, po, prs):
                    # fast PSUM->SBUF copies release the po/prs banks for the
                    # next phase (pop/prsp run with bufs=1)
                    rc = smallp.tile([P, 512], f32, tag="rc")
                    nc.vector.tensor_scalar_mul(out=rc[:], in0=prs[:], scalar1=1.0)
                    pc = smallp.tile([P, 512], f32, tag="pc")
                    nc.vector.tensor_scalar_mul(out=pc[:], in0=po[:], scalar1=1.0)
                    rcs = smallp.tile([P, 512], f32, tag="rcs")
                    scr = smallp.tile([P, 512], f32, tag="scr")
                    nc.vector.reciprocal_approx_accurate(rcs[:], rc[:], scr[:])
                    ob = smallp.tile([P, 512], f32, tag="ob")
                    nc.vector.tensor_tensor(ob[:], pc[:], rcs[:], Alu.mult)
                    nc.sync.dma_start(
                        out=out_d[pr * P : (pr + 1) * P, qh * QH : (qh + 1) * QH],
                        in_=ob[:],
                    )

                # prologue: K pair0 qb0 + Q pair0 qb0, dc-outer paired across the
                # two proj banks so both chains advance as each xT chunk lands
                psK = projp.tile([P, 512], f32, tag="proj")
                psQ = projp.tile([P, 512], f32, tag="proj")
                for dc in range(8):
                    qk_step(psK, 1, slice(0, 512), dc)
                    qk_step(psQ, 0, slice(0, 512), dc)
                nc.vector.tensor_scalar_add(
                    out=qk_all[:, 1, 0:512], in0=psK[:], scalar1=bqk_sb[:, 1:2]
                )
                nc.vector.tensor_scalar_add(
                    out=qk_all[:, 0, 0:512], in0=psQ[:], scalar1=bqk_sb[:, 0:1]
                )

                # remaining projection chains, emitted as paired dc-outer
                # BURSTS (two chains interleaved across the two proj banks: no
                # accumulate-dependency between consecutive matmuls and only
                # one weight-kind transition per burst). Keyed by super-step.
                # col: 0=Q pair0, 1=K pair0, 2=Q pair1, 3=K pair1
                burst_sched = {
                    0: [("qk", 1, 1)],
                    2: [("qk", 1, 2), ("qk", 1, 3)],
                    4: [("qk", 0, 1), ("v", 0)],
                    6: [("v", 1), ("v", 2)],
                    8: [("v", 3), ("v", 4)],
                    10: [("v", 5), ("v", 6)],
                    12: [("v", 7), ("v", 8)],
                    14: [("v", 9), ("v", 10)],
                    16: [("v", 11), ("v", 12)],
                    18: [("v", 13), ("v", 14)],
                    20: [("v", 15), ("qk", 0, 2)],
                    36: [("qk", 0, 3), ("qk", 3, 0)],
                    44: [("qk", 2, 0), ("qk", 3, 1)],
                    52: [("qk", 3, 2), ("qk", 3, 3)],
                    66: [("qk", 2, 1), ("qk", 2, 2)],
                    90: [("qk", 2, 3)],
                }

                def burst(specs):
                    gens = [
                        qk_chain(s[1], s[2]) if s[0] == "qk" else v_chain(s[1])
                        for s in specs
                    ]
                    alive = list(gens)
                    while alive:
                        for gn in list(alive):
                            try:
                                next(gn)
                            except StopIteration:
                                alive.remove(gn)

                es = {}
                cts = {}
                amults = {}
                bstate = {}
                NITER = NPH * NK
                for gg in range(0, NITER + BOFF, 2):
                    if gg < NITER:
                        ph = gg // NK
                        pr, qh = ph // 4, ph % 4
                        ks0, ks1 = gg % NK, gg % NK + 1
                        cts[gg] = ct_fetch(qh, ks0)
                        cts[gg + 1] = ct_fetch(qh, ks1)
                        ps0 = scores(pr, qh, ks0)
                        ps1 = scores(pr, qh, ks1)
                        es[gg] = expo(ps0)
                        es[gg + 1] = expo(ps1)
                        if gg in burst_sched:
                            burst(burst_sched[gg])
                    for ag in (gg - AOFF, gg - AOFF + 1):
                        if 0 <= ag < NITER:
                            amults[ag] = a_mult(es[ag], cts.pop(ag))
                    bg0 = gg - BOFF
                    if bg0 >= 0:
                        bph = bg0 // NK
                        bpr, bqh = bph // 4, bph % 4
                        bks0, bks1 = bg0 % NK, bg0 % NK + 1
                        if bks0 == 0:
                            b_po = pop.tile([P, 512], f32, tag="po")
                            b_prs = prsp.tile([P, 512], f32, tag="prs")
                            bstate[bph] = (b_po, b_prs)
                        b_po, b_prs = bstate[bph]
                        prs_mm(bks0, es[bg0], b_prs)
                        prs_mm(bks1, es[bg0 + 1], b_prs)
                        po_mm(bpr, bks0, amults[bg0], b_po)
                        po_mm(bpr, bks1, amults[bg0 + 1], b_po)
                        for bg in (bg0, bg0 + 1):
                            es.pop(bg)
                            amults.pop(bg)
                        if bks1 == NK - 1:
                            finishB(bpr, bqh, b_po, b_prs)
                            del bstate[bph]

    nc.finalize()
    return nc


def _get_program():
    if "nc" not in _prog_cache:
        _prog_cache["nc"] = _build_program()
    return _prog_cache["nc"]


def kernel(x, attention_mask, C_prior, Wq, bq, Wk, bk, Wv, bv):
    from concourse.bass_utils import run_bass_kernel_spmd

    x = np.asarray(x, dtype=np.float32)
    attention_mask = np.asarray(attention_mask)
    C_prior = np.asarray(C_prior, dtype=np.float32)
    Wq = np.asarray(Wq, dtype=np.float32)
    Wk = np.asarray(Wk, dtype=np.float32)
    Wv = np.asarray(Wv, dtype=np.float32)
    bq = np.asarray(bq, dtype=np.float32)
    bk = np.asarray(bk, dtype=np.float32)
    bv = np.asarray(bv, dtype=np.float32)
    bf = ml_dtypes.bfloat16

    WqT, WkT, WvT = Wq.T, Wk.T, Wv.T  # [in D, out D]
    maskf = attention_mask.astype(np.float32)  # [B, S]

    in_maps = []
    for c in range(NCORES):
        b, hg = c // 4, c % 4
        heads = [4 * hg + i for i in range(HEADS_PER_CORE)]
        xT = np.ascontiguousarray(x[b].T).astype(bf)  # [D, S]

        wqk = np.empty((D, 512), np.float32)
        bqk = np.zeros((P, 4), np.float32)
        for pr in range(2):
            h0, h1 = heads[2 * pr], heads[2 * pr + 1]
            wqk[:, (2 * pr) * P : (2 * pr) * P + 64] = WqT[:, h0 * 64 : h0 * 64 + 64]
            wqk[:, (2 * pr) * P + 64 : (2 * pr + 1) * P] = WqT[
                :, h1 * 64 : h1 * 64 + 64
            ]
            wqk[:, (2 * pr + 1) * P : (2 * pr + 1) * P + 64] = WkT[
                :, h0 * 64 : h0 * 64 + 64
            ]
            wqk[:, (2 * pr + 1) * P + 64 : (2 * pr + 2) * P] = WkT[
                :, h1 * 64 : h1 * 64 + 64
            ]
            bqk[0:64, 2 * pr] = bq[h0 * 64 : h0 * 64 + 64]
            bqk[64:128, 2 * pr] = bq[h1 * 64 : h1 * 64 + 64]
            bqk[0:64, 2 * pr + 1] = bk[h0 * 64 : h0 * 64 + 64]
            bqk[64:128, 2 * pr + 1] = bk[h1 * 64 : h1 * 64 + 64]

        wv = np.ascontiguousarray(
            WvT[:, heads[0] * 64 : (heads[-1] + 1) * 64]
        ).astype(bf)
        bvr = np.ascontiguousarray(
            np.broadcast_to(
                bv[heads[0] * 64 : (heads[-1] + 1) * 64][None, :], (P, 256)
            )
        )
        m = maskf[b]  # [S]
        ct = (C_prior[b].T * m[:, None]).astype(bf)  # [S(k), S(q)] * mask[k]
        mkcol = m.reshape(S // P, P).T.astype(bf)  # [P, 16]
        mk = np.ascontiguousarray(
            np.repeat(mkcol[:, :, None], 64, axis=2).reshape(P, -1)
        )  # [P, 16*64]

        in_maps.append(
            {
                "xT": xT,
                "wqk": wqk.astype(bf),
                "wv": wv,
                "bqk": bqk,
                "bvr": bvr,
                "ct": ct,
                "mk": mk,
            }
        )

    nc = _get_program()
    trace = bool(int(os.environ.get("BASS_KERNEL_TRACE", "0")))
    res = run_bass_kernel_spmd(nc, in_maps, list(range(NCORES)), trace=trace)
    if trace:
        print(f"HW exec time: {res.exec_time_ns} ns")
        _prog_cache["last_exec_time_ns"] = res.exec_time_ns
        _prog_cache["last_trace"] = res.instructions_and_trace

    out = np.empty((B, S, D), np.float32)
    for c in range(NCORES):
        b, hg = c // 4, c % 4
        co = res.results[c]["out"]  # [256, S]
        for i in range(HEADS_PER_CORE):
            h = 4 * hg + i
            out[b, :, h * 64 : (h + 1) * 64] = co[i * 64 : (i + 1) * 64, :].T
    return out


# revision 14
# speedup vs baseline: 1.0571x; 1.0571x over previous
"""Trainium2 Bass kernel for BertSelfAttention with C_prior multiply.

Reference (per batch b):
  q/k/v = x @ W{q,k,v}.T + b{q,k,v}            -> [S, D], split into H=16 heads of W=64
  scores = q k^T / sqrt(W); mask; softmax over k
  attn = softmax(scores) * C_prior[b]
  out = attn @ v                               -> [B, S, D]

Shapes: B=2, S=2048, D=1024, H=16, W=64.

Sharding: 8 cores; core c owns batch b=c//4 and 4 consecutive heads
(hg=c%4 -> heads 4*hg..4*hg+3). The whole per-(b,h) score block stays local.

Device pipeline (per core), v2:
  - 8 phases = (head pair pr in {0,1}) x (q block qh in {0..3} of 512).
  - Per k-strip (128 rows): the two heads of the pair are packed in the
    FREE dim of one [128, 1024] fp32 PSUM tile (2 banks): two row-tiled
    64-contraction matmuls run concurrently, then ONE 1024-wide EXP
    (halves the ScalarE instruction overhead -- ScalarE is the pipeline
    limiter at ~1 elem/cycle/lane).
  - attn*C: one 1024-wide VE bf16 multiply; the [128,512] ct strip is
    read twice via a stride-0 broadcast AP (no duplicate DMA).
  - Softmax denominator: ones(mask)-matmul pairs accumulated in one PSUM
    bank; A@V pairs in another. exp skips max-subtraction (scores~N(0,1)).
  - Projections are spread across the strip pipeline with a deadline
    schedule (one 8-matmul PSUM chain at a time in a dedicated bank);
    xT is DMA'd q-block-major so the first chains start after ~2MB.
  - Output O^T [w, q] written per phase; host transposes on gather.
"""

import os

import numpy as np
import ml_dtypes

B, S, D, H, W = 2, 2048, 1024, 16, 64
NCORES = 8
HEADS_PER_CORE = 4
P = 128
QH = 512  # q block per phase
NK = S // P  # 16 k-strips
NPH = 8  # phases: (pr, qh)
BOFF = 8  # pass B trails pass A by 8 strips
AOFF = 6  # attn*C multiply trails pass A by 6 strips

_prog_cache = {}


def _build_program():
    import concourse.mybir as mybir
    import concourse.tile as tile
    from concourse import bacc

    dt = mybir.dt
    f32, bf16 = dt.float32, dt.bfloat16
    Alu = mybir.AluOpType
    Act = mybir.ActivationFunctionType

    nc = bacc.Bacc("TRN2", target_bir_lowering=False)

    xT_d = nc.declare_dram_parameter("xT", [D, S], bf16, isOutput=False)
    wqk_d = nc.declare_dram_parameter("wqk", [D, 512], bf16, isOutput=False)
    wv_d = nc.declare_dram_parameter("wv", [D, 256], bf16, isOutput=False)
    bqk_d = nc.declare_dram_parameter("bqk", [P, 4], f32, isOutput=False)
    bvr_d = nc.declare_dram_parameter("bvr", [P, 256], f32, isOutput=False)
    ct_d = nc.declare_dram_parameter("ct", [S, S], bf16, isOutput=False)
    mk_d = nc.declare_dram_parameter("mk", [P, NK * 64], bf16, isOutput=False)
    out_d = nc.declare_dram_parameter("out", [256, S], f32, isOutput=True)

    with tile.TileContext(nc) as tc:
        with tc.tile_pool(name="persist", bufs=1) as persist:
            qk_all = persist.tile([P, 4, S], bf16)
            v_sb = persist.tile([P, NK, 256], bf16)
            bqk_sb = persist.tile([P, 4], f32)
            bvr_sb = persist.tile([P, 256], f32)
            mk_sb = persist.tile([P, NK, 64], bf16)
            xT_sb = persist.tile([P, 8, S], bf16)
            wqk_sb = persist.tile([P, 8, 512], bf16)
            wv_sb = persist.tile([P, 8, 256], bf16)
            nc.sync.dma_start(out=bqk_sb[:], in_=bqk_d[:])
            nc.sync.dma_start(out=bvr_sb[:], in_=bvr_d[:])
            nc.sync.dma_start(out=mk_sb[:], in_=mk_d[:])
            xT_r = xT_d.rearrange("(o p) q -> p o q", p=P)
            wqk_r = wqk_d.rearrange("(o p) m -> p o m", p=P)
            wv_r = wv_d.rearrange("(o p) m -> p o m", p=P)
            # critical first 2MB: wqk + xT q-block 0 (feeds the K0/Q0 chains);
            # keep >=1KB per-partition lines — finer splits are descriptor-bound
            for dc in range(8):
                nc.sync.dma_start(out=wqk_sb[:, dc, :], in_=wqk_r[:, dc, :])
                nc.sync.dma_start(out=xT_sb[:, dc, 0:512], in_=xT_r[:, dc, 0:512])
            for dc in range(8):
                nc.sync.dma_start(out=wv_sb[:, dc, :], in_=wv_r[:, dc, :])
            for qb in (1, 2, 3):
                for dc in range(8):
                    qs = slice(qb * 512, (qb + 1) * 512)
                    nc.sync.dma_start(out=xT_sb[:, dc, qs], in_=xT_r[:, dc, qs])

            with tc.tile_pool(name="estr", bufs=11) as ep, tc.tile_pool(
                name="astr", bufs=4
            ) as app, tc.tile_pool(name="ctp", bufs=9) as ctp, tc.tile_pool(
                name="small", bufs=1
            ) as smallp, tc.tile_pool(
                name="psA", bufs=2, space="PSUM"
            ) as psA, tc.tile_pool(
                name="pop", bufs=1, space="PSUM"
            ) as pop, tc.tile_pool(
                name="prsp", bufs=1, space="PSUM"
            ) as prsp, tc.tile_pool(
                name="projp", bufs=2, space="PSUM"
            ) as projp:

                def qk_step(ps, col, qs, dc):
                    # col-split: two concurrent 64-col quadrant matmuls whose
                    # weight loads overlap the neighbouring quadrant's stream
                    nc.tensor.matmul(
                        ps[0:64, :],
                        lhsT=wqk_sb[:, dc, col * P : col * P + 64],
                        rhs=xT_sb[:, dc, qs],
                        tile_position=(0, 0),
                        start=(dc == 0),
                        stop=(dc == 7),
                    )
                    nc.tensor.matmul(
                        ps[64:128, :],
                        lhsT=wqk_sb[:, dc, col * P + 64 : (col + 1) * P],
                        rhs=xT_sb[:, dc, qs],
                        tile_position=(0, 64),
                        start=(dc == 0),
                        stop=(dc == 7),
                    )

                def qk_chain(col, qb):
                    ps = projp.tile([P, 512], f32, tag="proj")
                    qs = slice(qb * 512, (qb + 1) * 512)
                    for dc in range(8):
                        qk_step(ps, col, qs, dc)
                        if dc < 7:
                            yield
                    nc.vector.tensor_scalar_add(
                        out=qk_all[:, col, qs],
                        in0=ps[:],
                        scalar1=bqk_sb[:, col : col + 1],
                    )

                def v_chain(kt):
                    ps = projp.tile([P, 512], f32, tag="proj")
                    for dc in range(8):
                        nc.tensor.matmul(
                            ps[0:64, 0:256],
                            lhsT=xT_sb[:, dc, kt * P : kt * P + 64],
                            rhs=wv_sb[:, dc, :],
                            tile_position=(0, 0),
                            start=(dc == 0),
                            stop=(dc == 7),
                        )
                        nc.tensor.matmul(
                            ps[64:128, 0:256],
                            lhsT=xT_sb[:, dc, kt * P + 64 : (kt + 1) * P],
                            rhs=wv_sb[:, dc, :],
                            tile_position=(0, 64),
                            start=(dc == 0),
                            stop=(dc == 7),
                        )
                        if dc < 7:
                            yield
                    nc.vector.tensor_tensor(
                        v_sb[:, kt, :], ps[:, 0:256], bvr_sb[:], Alu.add
                    )

                def ct_fetch(qh, ks):
                    ct = ctp.tile([P, 512], bf16, tag="ct")
                    nc.sync.dma_start(
                        out=ct[:],
                        in_=ct_d[ks * P : (ks + 1) * P, qh * QH : (qh + 1) * QH],
                    )
                    return ct

                def scores(pr, qh, ks):
                    # 4 quadrant matmuls (row x col split), all concurrent
                    pss = psA.tile([P, 1024], f32, tag="scT")
                    qs = slice(qh * QH, (qh + 1) * QH)
                    for hh, rlo in ((0, 0), (1, 64)):
                        rs = slice(rlo, rlo + 64)
                        os_ = slice(hh * 512, (hh + 1) * 512)
                        nc.tensor.matmul(
                            pss[0:64, os_],
                            lhsT=qk_all[rs, 2 * pr + 1, ks * P : ks * P + 64],
                            rhs=qk_all[rs, 2 * pr, qs],
                            tile_position=(rlo, 0),
                            start=True,
                            stop=True,
                        )
                        nc.tensor.matmul(
                            pss[64:128, os_],
                            lhsT=qk_all[rs, 2 * pr + 1, ks * P + 64 : (ks + 1) * P],
                            rhs=qk_all[rs, 2 * pr, qs],
                            tile_position=(rlo, 64),
                            start=True,
                            stop=True,
                        )
                    return pss

                def expo(pss):
                    e = ep.tile([P, 1024], bf16, tag="e")
                    nc.scalar.activation(e[:], pss[:], Act.Exp, scale=0.125)
                    return e

                def a_mult(e, ct):
                    a = app.tile([P, 1024], bf16, tag="a")
                    ct_b = ct[:, None, :].broadcast_to([P, 2, 512])
                    nc.vector.tensor_tensor(
                        a[:].rearrange("p (r f) -> p r f", r=2),
                        e[:].rearrange("p (r f) -> p r f", r=2),
                        ct_b,
                        Alu.mult,
                    )
                    return a

                def prs_mm(ks, e, prs):
                    st, sp = (ks == 0), (ks == NK - 1)
                    nc.tensor.matmul(
                        prs[0:64, :],
                        lhsT=mk_sb[:, ks, :],
                        rhs=e[:, 0:512],
                        tile_position=(0, 0),
                        start=st,
                        stop=sp,
                    )
                    nc.tensor.matmul(
                        prs[64:128, :],
                        lhsT=mk_sb[:, ks, :],
                        rhs=e[:, 512:1024],
                        tile_position=(0, 64),
                        start=st,
                        stop=sp,
                    )

                def po_mm(pr, ks, a, po):
                    st, sp = (ks == 0), (ks == NK - 1)
                    h0, h1 = 2 * pr, 2 * pr + 1
                    nc.tensor.matmul(
                        po[0:64, :],
                        lhsT=v_sb[:, ks, h0 * 64 : (h0 + 1) * 64],
                        rhs=a[:, 0:512],
                        tile_position=(0, 0),
                        start=st,
                        stop=sp,
                    )
                    nc.tensor.matmul(
                        po[64:128, :],
                        lhsT=v_sb[:, ks, h1 * 64 : (h1 + 1) * 64],
                        rhs=a[:, 512:1024],
                        tile_position=(0, 64),
                        start=st,
                        stop=sp,
                    )

                def finishB(pr, qh, po, prs):
                    # fast PSUM->SBUF copies release the po/prs banks for the
                    # next phase (pop/prsp run with bufs=1)
                    rc = smallp.tile([P, 512], f32, tag="rc")
                    nc.vector.tensor_scalar_mul(out=rc[:], in0=prs[:], scalar1=1.0)
                    pc = smallp.tile([P, 512], f32, tag="pc")
                    nc.vector.tensor_scalar_mul(out=pc[:], in0=po[:], scalar1=1.0)
                    rcs = smallp.tile([P, 512], f32, tag="rcs")
                    scr = smallp.tile([P, 512], f32, tag="scr")
                    nc.vector.reciprocal_approx_accurate(rcs[:], rc[:], scr[:])
                    ob = smallp.tile([P, 512], f32, tag="ob")
                    nc.vector.tensor_tensor(ob[:], pc[:], rcs[:], Alu.mult)
                    nc.sync.dma_start(
                        out=out_d[pr * P : (pr + 1) * P, qh * QH : (qh + 1) * QH],
                        in_=ob[:],
                    )

                # prologue: K pair0 qb0 + Q pair0 qb0, dc-outer paired across the
                # two proj banks so both chains advance as each xT chunk lands
                psK = projp.tile([P, 512], f32, tag="proj")
                psQ = projp.tile([P, 512], f32, tag="proj")
                for dc in range(8):
                    qk_step(psK, 1, slice(0, 512), dc)
                    qk_step(psQ, 0, slice(0, 512), dc)
                nc.vector.tensor_scalar_add(
                    out=qk_all[:, 1, 0:512], in0=psK[:], scalar1=bqk_sb[:, 1:2]
                )
                nc.vector.tensor_scalar_add(
                    out=qk_all[:, 0, 0:512], in0=psQ[:], scalar1=bqk_sb[:, 0:1]
                )

                # remaining projection chains, emitted as paired dc-outer
                # BURSTS (two chains interleaved across the two proj banks: no
                # accumulate-dependency between consecutive matmuls and only
                # one weight-kind transition per burst). Keyed by super-step.
                # col: 0=Q pair0, 1=K pair0, 2=Q pair1, 3=K pair1
                burst_sched = {
                    0: [("qk", 1, 1)],
                    2: [("qk", 1, 2), ("qk", 1, 3)],
                    4: [("qk", 0, 1), ("v", 0)],
                    6: [("v", 1), ("v", 2)],
                    8: [("v", 3), ("v", 4)],
                    10: [("v", 5), ("v", 6)],
                    12: [("v", 7), ("v", 8)],
                    14: [("v", 9), ("v", 10)],
                    16: [("v", 11), ("v", 12)],
                    18: [("v", 13), ("v", 14)],
                    20: [("v", 15), ("qk", 0, 2)],
                    36: [("qk", 0, 3)],
                    38: [("qk", 3, 0)],
                    44: [("qk", 2, 0)],
                    46: [("qk", 3, 1)],
                    52: [("qk", 3, 2)],
                    54: [("qk", 3, 3)],
                    66: [("qk", 2, 1)],
                    68: [("qk", 2, 2)],
                    90: [("qk", 2, 3)],
                }

                def burst(specs):
                    gens = [
                        qk_chain(s[1], s[2]) if s[0] == "qk" else v_chain(s[1])
                        for s in specs
                    ]
                    alive = list(gens)
                    while alive:
                        for gn in list(alive):
                            try:
                                next(gn)
                            except StopIteration:
                                alive.remove(gn)

                es = {}
                cts = {}
                amults = {}
                bstate = {}
                NITER = NPH * NK
                for gg in range(0, NITER + BOFF, 2):
                    if gg < NITER:
                        ph = gg // NK
                        pr, qh = ph // 4, ph % 4
                        ks0, ks1 = gg % NK, gg % NK + 1
                        cts[gg] = ct_fetch(qh, ks0)
                        cts[gg + 1] = ct_fetch(qh, ks1)
                        ps0 = scores(pr, qh, ks0)
                        ps1 = scores(pr, qh, ks1)
                        es[gg] = expo(ps0)
                        es[gg + 1] = expo(ps1)
                        if gg in burst_sched:
                            burst(burst_sched[gg])
                    for ag in (gg - AOFF, gg - AOFF + 1):
                        if 0 <= ag < NITER:
                            amults[ag] = a_mult(es[ag], cts.pop(ag))
                    bg0 = gg - BOFF
                    if bg0 >= 0:
                        bph = bg0 // NK
                        bpr, bqh = bph // 4, bph % 4
                        bks0, bks1 = bg0 % NK, bg0 % NK + 1
                        if bks0 == 0:
                            b_po = pop.tile([P, 512], f32, tag="po")
                            b_prs = prsp.tile([P, 512], f32, tag="prs")
                            bstate[bph] = (b_po, b_prs)
                        b_po, b_prs = bstate[bph]
                        prs_mm(bks0, es[bg0], b_prs)
                        prs_mm(bks1, es[bg0 + 1], b_prs)
                        po_mm(bpr, bks0, amults[bg0], b_po)
                        po_mm(bpr, bks1, amults[bg0 + 1], b_po)
                        for bg in (bg0, bg0 + 1):
                            es.pop(bg)
                            amults.pop(bg)
                        if bks1 == NK - 1:
                            finishB(bpr, bqh, b_po, b_prs)
                            del bstate[bph]

    nc.finalize()
    return nc


def _get_program():
    if "nc" not in _prog_cache:
        _prog_cache["nc"] = _build_program()
    return _prog_cache["nc"]


def kernel(x, attention_mask, C_prior, Wq, bq, Wk, bk, Wv, bv):
    from concourse.bass_utils import run_bass_kernel_spmd

    x = np.asarray(x, dtype=np.float32)
    attention_mask = np.asarray(attention_mask)
    C_prior = np.asarray(C_prior, dtype=np.float32)
    Wq = np.asarray(Wq, dtype=np.float32)
    Wk = np.asarray(Wk, dtype=np.float32)
    Wv = np.asarray(Wv, dtype=np.float32)
    bq = np.asarray(bq, dtype=np.float32)
    bk = np.asarray(bk, dtype=np.float32)
    bv = np.asarray(bv, dtype=np.float32)
    bf = ml_dtypes.bfloat16

    WqT, WkT, WvT = Wq.T, Wk.T, Wv.T  # [in D, out D]
    maskf = attention_mask.astype(np.float32)  # [B, S]

    in_maps = []
    for c in range(NCORES):
        b, hg = c // 4, c % 4
        heads = [4 * hg + i for i in range(HEADS_PER_CORE)]
        xT = np.ascontiguousarray(x[b].T).astype(bf)  # [D, S]

        wqk = np.empty((D, 512), np.float32)
        bqk = np.zeros((P, 4), np.float32)
        for pr in range(2):
            h0, h1 = heads[2 * pr], heads[2 * pr + 1]
            wqk[:, (2 * pr) * P : (2 * pr) * P + 64] = WqT[:, h0 * 64 : h0 * 64 + 64]
            wqk[:, (2 * pr) * P + 64 : (2 * pr + 1) * P] = WqT[
                :, h1 * 64 : h1 * 64 + 64
            ]
            wqk[:, (2 * pr + 1) * P : (2 * pr + 1) * P + 64] = WkT[
                :, h0 * 64 : h0 * 64 + 64
            ]
            wqk[:, (2 * pr + 1) * P + 64 : (2 * pr + 2) * P] = WkT[
                :, h1 * 64 : h1 * 64 + 64
            ]
            bqk[0:64, 2 * pr] = bq[h0 * 64 : h0 * 64 + 64]
            bqk[64:128, 2 * pr] = bq[h1 * 64 : h1 * 64 + 64]
            bqk[0:64, 2 * pr + 1] = bk[h0 * 64 : h0 * 64 + 64]
            bqk[64:128, 2 * pr + 1] = bk[h1 * 64 : h1 * 64 + 64]

        wv = np.ascontiguousarray(
            WvT[:, heads[0] * 64 : (heads[-1] + 1) * 64]
        ).astype(bf)
        bvr = np.ascontiguousarray(
            np.broadcast_to(
                bv[heads[0] * 64 : (heads[-1] + 1) * 64][None, :], (P, 256)
            )
        )
        m = maskf[b]  # [S]
        ct = (C_prior[b].T * m[:, None]).astype(bf)  # [S(k), S(q)] * mask[k]
        mkcol = m.reshape(S // P, P).T.astype(bf)  # [P, 16]
        mk = np.ascontiguousarray(
            np.repeat(mkcol[:, :, None], 64, axis=2).reshape(P, -1)
        )  # [P, 16*64]

        in_maps.append(
            {
                "xT": xT,
                "wqk": wqk.astype(bf),
                "wv": wv,
                "bqk": bqk,
                "bvr": bvr,
                "ct": ct,
                "mk": mk,
            }
        )

    nc = _get_program()
    trace = bool(int(os.environ.get("BASS_KERNEL_TRACE", "0")))
    res = run_bass_kernel_spmd(nc, in_maps, list(range(NCORES)), trace=trace)
    if trace:
        print(f"HW exec time: {res.exec_time_ns} ns")
        _prog_cache["last_exec_time_ns"] = res.exec_time_ns
        _prog_cache["last_trace"] = res.instructions_and_trace

    out = np.empty((B, S, D), np.float32)
    for c in range(NCORES):
        b, hg = c // 4, c % 4
        co = res.results[c]["out"]  # [256, S]
        for i in range(HEADS_PER_CORE):
            h = 4 * hg + i
            out[b, :, h * 64 : (h + 1) * 64] = co[i * 64 : (i + 1) * 64, :].T
    return out


# revision 17
# speedup vs baseline: 1.0814x; 1.0230x over previous
"""Trainium2 Bass kernel for BertSelfAttention with C_prior multiply.

Reference (per batch b):
  q/k/v = x @ W{q,k,v}.T + b{q,k,v}            -> [S, D], split into H=16 heads of W=64
  scores = q k^T / sqrt(W); mask; softmax over k
  attn = softmax(scores) * C_prior[b]
  out = attn @ v                               -> [B, S, D]

Shapes: B=2, S=2048, D=1024, H=16, W=64.

Sharding: 8 cores; core c owns batch b=c//4 and 4 consecutive heads
(hg=c%4 -> heads 4*hg..4*hg+3). The whole per-(b,h) score block stays local.

Device pipeline (per core), v2:
  - 8 phases = (head pair pr in {0,1}) x (q block qh in {0..3} of 512).
  - Per k-strip (128 rows): the two heads of the pair are packed in the
    FREE dim of one [128, 1024] fp32 PSUM tile (2 banks): two row-tiled
    64-contraction matmuls run concurrently, then ONE 1024-wide EXP
    (halves the ScalarE instruction overhead -- ScalarE is the pipeline
    limiter at ~1 elem/cycle/lane).
  - attn*C: one 1024-wide VE bf16 multiply; the [128,512] ct strip is
    read twice via a stride-0 broadcast AP (no duplicate DMA).
  - Softmax denominator: ones(mask)-matmul pairs accumulated in one PSUM
    bank; A@V pairs in another. exp skips max-subtraction (scores~N(0,1)).
  - Projections are spread across the strip pipeline with a deadline
    schedule (one 8-matmul PSUM chain at a time in a dedicated bank);
    xT is DMA'd q-block-major so the first chains start after ~2MB.
  - Output O^T [w, q] written per phase; host transposes on gather.
"""

import os

import numpy as np
import ml_dtypes

B, S, D, H, W = 2, 2048, 1024, 16, 64
NCORES = 8
HEADS_PER_CORE = 4
P = 128
QH = 512  # q block per phase
NK = S // P  # 16 k-strips
NPH = 8  # phases: (pr, qh)
BOFF = 8  # pass B trails pass A by 8 strips
AOFF = 6  # attn*C multiply trails pass A by 6 strips

_prog_cache = {}


def _build_program():
    import concourse.mybir as mybir
    import concourse.tile as tile
    from concourse import bacc

    dt = mybir.dt
    f32, bf16 = dt.float32, dt.bfloat16
    Alu = mybir.AluOpType
    Act = mybir.ActivationFunctionType

    nc = bacc.Bacc("TRN2", target_bir_lowering=False)

    xT_d = nc.declare_dram_parameter("xT", [D, S], bf16, isOutput=False)
    wqk_d = nc.declare_dram_parameter("wqk", [D, 512], bf16, isOutput=False)
    wv_d = nc.declare_dram_parameter("wv", [D, 256], bf16, isOutput=False)
    bqk_d = nc.declare_dram_parameter("bqk", [P, 4], f32, isOutput=False)
    bvr_d = nc.declare_dram_parameter("bvr", [P, 256], f32, isOutput=False)
    ct_d = nc.declare_dram_parameter("ct", [S, S], bf16, isOutput=False)
    mk_d = nc.declare_dram_parameter("mk", [P, NK * 64], bf16, isOutput=False)
    out_d = nc.declare_dram_parameter("out", [256, S], f32, isOutput=True)

    with tile.TileContext(nc) as tc:
        with tc.tile_pool(name="persist", bufs=1) as persist:
            qk_all = persist.tile([P, 4, S], bf16)
            v_sb = persist.tile([P, NK, 256], bf16)
            bqk_sb = persist.tile([P, 4], f32)
            bvr_sb = persist.tile([P, 256], f32)
            mk_sb = persist.tile([P, NK, 64], bf16)
            xT_sb = persist.tile([P, 8, S], bf16)
            wqk_sb = persist.tile([P, 8, 512], bf16)
            wv_sb = persist.tile([P, 8, 256], bf16)
            nc.sync.dma_start(out=bqk_sb[:], in_=bqk_d[:])
            nc.sync.dma_start(out=bvr_sb[:], in_=bvr_d[:])
            nc.sync.dma_start(out=mk_sb[:], in_=mk_d[:])
            xT_r = xT_d.rearrange("(o p) q -> p o q", p=P)
            wqk_r = wqk_d.rearrange("(o p) m -> p o m", p=P)
            wv_r = wv_d.rearrange("(o p) m -> p o m", p=P)
            # critical first 2MB: wqk + xT q-block 0 (feeds the K0/Q0 chains)
            for dc in range(8):
                nc.sync.dma_start(out=wqk_sb[:, dc, :], in_=wqk_r[:, dc, :])
                nc.sync.dma_start(out=xT_sb[:, dc, 0:512], in_=xT_r[:, dc, 0:512])
            for dc in range(8):
                nc.sync.dma_start(out=wv_sb[:, dc, :], in_=wv_r[:, dc, :])
            for qb in (1, 2, 3):
                for dc in range(8):
                    qs = slice(qb * 512, (qb + 1) * 512)
                    nc.sync.dma_start(out=xT_sb[:, dc, qs], in_=xT_r[:, dc, qs])

            with tc.tile_pool(name="estr", bufs=11) as ep, tc.tile_pool(
                name="astr", bufs=4
            ) as app, tc.tile_pool(name="ctp", bufs=9) as ctp, tc.tile_pool(
                name="small", bufs=1
            ) as smallp, tc.tile_pool(
                name="psA", bufs=2, space="PSUM"
            ) as psA, tc.tile_pool(
                name="pop", bufs=2, space="PSUM"
            ) as pop, tc.tile_pool(
                name="prsp", bufs=1, space="PSUM"
            ) as prsp, tc.tile_pool(
                name="projp", bufs=1, space="PSUM"
            ) as projp:

                def proj_qk(col, qb):
                    ps = projp.tile([P, 512], f32, tag="proj")
                    qs = slice(qb * 512, (qb + 1) * 512)
                    for dc in range(8):
                        nc.tensor.matmul(
                            ps[:],
                            lhsT=wqk_sb[:, dc, col * P : (col + 1) * P],
                            rhs=xT_sb[:, dc, qs],
                            start=(dc == 0),
                            stop=(dc == 7),
                        )
                    nc.vector.tensor_scalar_add(
                        out=qk_all[:, col, qs],
                        in0=ps[:],
                        scalar1=bqk_sb[:, col : col + 1],
                    )

                def proj_v(kt):
                    ps = projp.tile([P, 512], f32, tag="proj")
                    for dc in range(8):
                        nc.tensor.matmul(
                            ps[:, 0:256],
                            lhsT=xT_sb[:, dc, kt * P : (kt + 1) * P],
                            rhs=wv_sb[:, dc, :],
                            start=(dc == 0),
                            stop=(dc == 7),
                        )
                    nc.vector.tensor_tensor(
                        v_sb[:, kt, :], ps[:, 0:256], bvr_sb[:], Alu.add
                    )

                def ct_fetch(qh, ks):
                    ct = ctp.tile([P, 512], bf16, tag="ct")
                    nc.sync.dma_start(
                        out=ct[:],
                        in_=ct_d[ks * P : (ks + 1) * P, qh * QH : (qh + 1) * QH],
                    )
                    return ct

                def passA(pr, qh, ks):
                    pss = psA.tile([P, 1024], f32, tag="scT")
                    qs = slice(qh * QH, (qh + 1) * QH)
                    nc.tensor.matmul(
                        pss[:, 0:512],
                        lhsT=qk_all[0:64, 2 * pr + 1, ks * P : (ks + 1) * P],
                        rhs=qk_all[0:64, 2 * pr, qs],
                        tile_position=(0, 0),
                        start=True,
                        stop=True,
                    )
                    nc.tensor.matmul(
                        pss[:, 512:1024],
                        lhsT=qk_all[64:128, 2 * pr + 1, ks * P : (ks + 1) * P],
                        rhs=qk_all[64:128, 2 * pr, qs],
                        tile_position=(64, 0),
                        start=True,
                        stop=True,
                    )
                    e = ep.tile([P, 1024], bf16, tag="e")
                    nc.scalar.activation(e[:], pss[:], Act.Exp, scale=0.125)
                    return e

                def a_mult(e, ct):
                    a = app.tile([P, 1024], bf16, tag="a")
                    ct_b = ct[:, None, :].broadcast_to([P, 2, 512])
                    nc.vector.tensor_tensor(
                        a[:].rearrange("p (r f) -> p r f", r=2),
                        e[:].rearrange("p (r f) -> p r f", r=2),
                        ct_b,
                        Alu.mult,
                    )
                    return a

                def prs_mm(ks, e, prs):
                    st, sp = (ks == 0), (ks == NK - 1)
                    nc.tensor.matmul(
                        prs[0:64, :],
                        lhsT=mk_sb[:, ks, :],
                        rhs=e[:, 0:512],
                        tile_position=(0, 0),
                        start=st,
                        stop=sp,
                    )
                    nc.tensor.matmul(
                        prs[64:128, :],
                        lhsT=mk_sb[:, ks, :],
                        rhs=e[:, 512:1024],
                        tile_position=(0, 64),
                        start=st,
                        stop=sp,
                    )

                def po_mm(pr, ks, a, po):
                    st, sp = (ks == 0), (ks == NK - 1)
                    h0, h1 = 2 * pr, 2 * pr + 1
                    nc.tensor.matmul(
                        po[0:64, :],
                        lhsT=v_sb[:, ks, h0 * 64 : (h0 + 1) * 64],
                        rhs=a[:, 0:512],
                        tile_position=(0, 0),
                        start=st,
                        stop=sp,
                    )
                    nc.tensor.matmul(
                        po[64:128, :],
                        lhsT=v_sb[:, ks, h1 * 64 : (h1 + 1) * 64],
                        rhs=a[:, 512:1024],
                        tile_position=(0, 64),
                        start=st,
                        stop=sp,
                    )

                def finishB(pr, qh, po, prs):
                    # fast PSUM->SBUF copy releases the prs bank early
                    rc = smallp.tile([P, 512], f32, tag="rc")
                    nc.vector.tensor_scalar_mul(out=rc[:], in0=prs[:], scalar1=1.0)
                    rcs = smallp.tile([P, 512], f32, tag="rcs")
                    scr = smallp.tile([P, 512], f32, tag="scr")
                    nc.vector.reciprocal_approx_accurate(rcs[:], rc[:], scr[:])
                    ob = smallp.tile([P, 512], f32, tag="ob")
                    nc.vector.tensor_tensor(ob[:], po[:], rcs[:], Alu.mult)
                    nc.sync.dma_start(
                        out=out_d[pr * P : (pr + 1) * P, qh * QH : (qh + 1) * QH],
                        in_=ob[:],
                    )

                # projection chain schedule: iteration -> chain
                # col: 0=Q pair0, 1=K pair0, 2=Q pair1, 3=K pair1
                chain_sched = {
                    0: ("qk", 1, 1),
                    1: ("v", 0),
                    2: ("qk", 1, 2),
                    3: ("v", 1),
                    4: ("qk", 1, 3),
                    5: ("v", 2),
                    6: ("v", 3),
                    7: ("v", 4),
                    8: ("v", 5),
                    9: ("v", 6),
                    10: ("v", 7),
                    11: ("v", 8),
                    12: ("qk", 0, 1),
                    13: ("v", 9),
                    14: ("v", 10),
                    15: ("v", 11),
                    16: ("v", 12),
                    17: ("v", 13),
                    18: ("v", 14),
                    19: ("v", 15),
                    28: ("qk", 0, 2),
                    36: ("qk", 0, 3),
                    40: ("qk", 3, 0),
                    44: ("qk", 3, 1),
                    48: ("qk", 3, 2),
                    52: ("qk", 3, 3),
                    56: ("qk", 2, 0),
                    72: ("qk", 2, 1),
                    88: ("qk", 2, 2),
                    104: ("qk", 2, 3),
                }

                # prologue: K pair0 qb0 + Q pair0 qb0
                proj_qk(1, 0)
                proj_qk(0, 0)

                es = {}
                cts = {}
                amults = {}
                bstate = {}
                NITER = NPH * NK
                for g in range(NITER + BOFF):
                    if g < NITER:
                        ph, ks = g // NK, g % NK
                        pr, qh = ph // 4, ph % 4
                        cts[g] = ct_fetch(qh, ks)
                        es[g] = passA(pr, qh, ks)
                        c = chain_sched.get(g)
                        if c is not None:
                            if c[0] == "qk":
                                proj_qk(c[1], c[2])
                            else:
                                proj_v(c[1])
                    ag = g - AOFF
                    if 0 <= ag < NITER:
                        amults[ag] = a_mult(es[ag], cts.pop(ag))
                    bg = g - BOFF
                    if bg >= 0 and bg % 2 == 1:
                        # 2-strip batched pass B: same-kind matmul pairs
                        # back-to-back so weight loads overlap streaming
                        bg0 = bg - 1
                        bph, bks0 = bg0 // NK, bg0 % NK
                        bpr, bqh = bph // 4, bph % 4
                        if bks0 == 0:
                            b_po = pop.tile([P, 512], f32, tag="po")
                            b_prs = prsp.tile([P, 512], f32, tag="prs")
                            bstate[bph] = (b_po, b_prs)
                        b_po, b_prs = bstate[bph]
                        prs_mm(bks0, es[bg0], b_prs)
                        prs_mm(bks0 + 1, es[bg0 + 1], b_prs)
                        po_mm(bpr, bks0, amults[bg0], b_po)
                        po_mm(bpr, bks0 + 1, amults[bg0 + 1], b_po)
                        for b_ in (bg0, bg0 + 1):
                            es.pop(b_)
                            amults.pop(b_)
                        if bks0 + 1 == NK - 1:
                            finishB(bpr, bqh, b_po, b_prs)
                            del bstate[bph]

    nc.finalize()
    return nc


def _get_program():
    if "nc" not in _prog_cache:
        _prog_cache["nc"] = _build_program()
    return _prog_cache["nc"]


def kernel(x, attention_mask, C_prior, Wq, bq, Wk, bk, Wv, bv):
    from concourse.bass_utils import run_bass_kernel_spmd

    x = np.asarray(x, dtype=np.float32)
    attention_mask = np.asarray(attention_mask)
    C_prior = np.asarray(C_prior, dtype=np.float32)
    Wq = np.asarray(Wq, dtype=np.float32)
    Wk = np.asarray(Wk, dtype=np.float32)
    Wv = np.asarray(Wv, dtype=np.float32)
    bq = np.asarray(bq, dtype=np.float32)
    bk = np.asarray(bk, dtype=np.float32)
    bv = np.asarray(bv, dtype=np.float32)
    bf = ml_dtypes.bfloat16

    WqT, WkT, WvT = Wq.T, Wk.T, Wv.T  # [in D, out D]
    maskf = attention_mask.astype(np.float32)  # [B, S]

    in_maps = []
    for c in range(NCORES):
        b, hg = c // 4, c % 4
        heads = [4 * hg + i for i in range(HEADS_PER_CORE)]
        xT = np.ascontiguousarray(x[b].T).astype(bf)  # [D, S]

        wqk = np.empty((D, 512), np.float32)
        bqk = np.zeros((P, 4), np.float32)
        for pr in range(2):
            h0, h1 = heads[2 * pr], heads[2 * pr + 1]
            wqk[:, (2 * pr) * P : (2 * pr) * P + 64] = WqT[:, h0 * 64 : h0 * 64 + 64]
            wqk[:, (2 * pr) * P + 64 : (2 * pr + 1) * P] = WqT[
                :, h1 * 64 : h1 * 64 + 64
            ]
            wqk[:, (2 * pr + 1) * P : (2 * pr + 1) * P + 64] = WkT[
                :, h0 * 64 : h0 * 64 + 64
            ]
            wqk[:, (2 * pr + 1) * P + 64 : (2 * pr + 2) * P] = WkT[
                :, h1 * 64 : h1 * 64 + 64
            ]
            bqk[0:64, 2 * pr] = bq[h0 * 64 : h0 * 64 + 64]
            bqk[64:128, 2 * pr] = bq[h1 * 64 : h1 * 64 + 64]
            bqk[0:64, 2 * pr + 1] = bk[h0 * 64 : h0 * 64 + 64]
            bqk[64:128, 2 * pr + 1] = bk[h1 * 64 : h1 * 64 + 64]

        wv = np.ascontiguousarray(
            WvT[:, heads[0] * 64 : (heads[-1] + 1) * 64]
        ).astype(bf)
        bvr = np.ascontiguousarray(
            np.broadcast_to(
                bv[heads[0] * 64 : (heads[-1] + 1) * 64][None, :], (P, 256)
            )
        )
        m = maskf[b]  # [S]
        ct = (C_prior[b].T * m[:, None]).astype(bf)  # [S(k), S(q)] * mask[k]
        mkcol = m.reshape(S // P, P).T.astype(bf)  # [P, 16]
        mk = np.ascontiguousarray(
            np.repeat(mkcol[:, :, None], 64, axis=2).reshape(P, -1)
        )  # [P, 16*64]

        in_maps.append(
            {
                "xT": xT,
                "wqk": wqk.astype(bf),
                "wv": wv,
                "bqk": bqk,
                "bvr": bvr,
                "ct": ct,
                "mk": mk,
            }
        )

    nc = _get_program()
    trace = bool(int(os.environ.get("BASS_KERNEL_TRACE", "0")))
    res = run_bass_kernel_spmd(nc, in_maps, list(range(NCORES)), trace=trace)
    if trace:
        print(f"HW exec time: {res.exec_time_ns} ns")
        _prog_cache["last_exec_time_ns"] = res.exec_time_ns
        _prog_cache["last_trace"] = res.instructions_and_trace

    out = np.empty((B, S, D), np.float32)
    for c in range(NCORES):
        b, hg = c // 4, c % 4
        co = res.results[c]["out"]  # [256, S]
        for i in range(HEADS_PER_CORE):
            h = 4 * hg + i
            out[b, :, h * 64 : (h + 1) * 64] = co[i * 64 : (i + 1) * 64, :].T
    return out
